# revision 6
# baseline (speedup 1.0000x reference)
"""LocalExpansion (7x7 unfold) Trainium2 Bass kernel — v2.

Full input x: [2, 8, 2304, 64] f32; output [2, 8, 2304, 49, 64] f32:
out[b,h,y*48+x,i*7+j,:] = img[b,h,y+i-3,x+j-3,:], zero outside.

16 images, 2 per core. v1 (pure DMA from padded rows) hit 345us: the
output's contiguous runs per (y,x,i) are only 1792 B, and ~32K small
descriptors at ~128 ns each cap the 16 SDMA engines at ~14 GB/s.

v2 buys big descriptors with a staged pipeline:
1. Load the image 7x from HBM (once per filter row i), each copy
   placed so padded row y+i sits on partition y (compute engines
   cannot read partition-shifted, so the shift is baked into the
   load). Extra HBM reads: 8.3 MB vs 57.8 MB of writes.
2. Compute engines (vector+scalar+gpsimd) build output-layout tiles:
   ob[y][x'*3136 + i*448 + j*64 + d] <- pads_i[y][(x+j)*64+d] — a
   free-dim overlapping-window copy, both images at once on 112
   partitions (imgA rows on 0-47, imgB on 64-111).
3. Out-DMA per x-block: one descriptor per partition = 3*49*64 f32
   = 37.6 KB contiguous both sides -> near line rate, HBM-bound.
Double-buffered x-blocks (XB=3, 16 blocks) overlap 2 and 3.
"""

import numpy as np

KH, KW = 7, 7
H, W, D = 48, 48, 64
N = H * W                      # 2304
K = KH * KW                    # 49
IMG_OUT = N * K * D            # 7,225,344 floats per image output
ROWI = (W + 6) * D             # 3456 floats per padded row
PCOL = KH * ROWI               # 24192 floats/partition in "pads"
XB = 3                         # x positions per output block
NB = W // XB                   # 16 blocks
OBH = XB * K * D               # 9408 floats: one buffer half per partition
OBROW = 2 * OBH                # 18816 floats/partition in "ob"
IMGS_PER_CORE = 2
N_CORES = 8
BASES = (0, 64)                # partition bases per image

_CACHE = {}


def _build_nc():
    import concourse.bass as bass
    import concourse.mybir as mybir

    nc = bass.Bass(trn_type="TRN2")
    x = nc.dram_tensor("x", [IMGS_PER_CORE, N, D], mybir.dt.float32,
                       kind="ExternalInput")
    out = nc.dram_tensor("out", [IMGS_PER_CORE, N, K, D], mybir.dt.float32,
                         kind="ExternalOutput")

    with (
        nc.sbuf_tensor("pads", [128, PCOL], mybir.dt.float32) as pads,
        nc.sbuf_tensor("ob", [128, OBROW], mybir.dt.float32) as ob,
        nc.semaphore("ms") as ms,
        nc.semaphore("ld") as ld,
        nc.semaphore("cpv") as cpv,
        nc.semaphore("cps") as cps,
        nc.semaphore("cpg") as cpg,
        nc.semaphore("st0") as st0,
        nc.semaphore("st1") as st1,
    ):
        # Zero "pads" (pad strips + junk partitions 48-63 read as 0).
        # Split across the three copy engines.
        third = PCOL // 3
        nc.vector.memset(bass.AP(pads, 0, [[PCOL, 128], [1, third]]), 0.0
                         ).then_inc(ms, 1)
        nc.gpsimd.memset(bass.AP(pads, third, [[PCOL, 128], [1, third]]), 0.0
                         ).then_inc(ms, 1)
        nc.vector.memset(bass.AP(pads, 2 * third, [[PCOL, 128], [1, third]]),
                         0.0).then_inc(ms, 1)

        # 14 shifted loads: pads[:, i-block] holds padded rows y+i on
        # partition y. Image rows g land on partition g+3-i.
        nc.sync.wait_ge(ms, 3)
        for im in range(IMGS_PER_CORE):
            bp = BASES[im]
            for i in range(KH):
                g_lo = max(0, i - 3)
                g_hi = min(H, i + 45)
                n_g = g_hi - g_lo
                p0 = g_lo + 3 - i
                nc.sync.dma_start(
                    out=bass.AP(
                        pads,
                        (bp + p0) * PCOL + i * ROWI + 3 * D,
                        [[PCOL, n_g], [1, W * D]],
                    ),
                    in_=bass.AP(
                        x,
                        im * N * D + g_lo * W * D,
                        [[W * D, n_g], [1, W * D]],
                    ),
                ).then_inc(ld, 16)
        n_ld = IMGS_PER_CORE * KH * 16

        # copy i -> engine: vector {0,3,6}, scalar {1,4}, gpsimd {2,5}
        def _copy(eng, b, h, i):
            out_ap = bass.AP(
                ob,
                h * OBH + i * KW * D,
                [[OBROW, 112], [K * D, XB], [D, KH], [1, D]],
            )
            in_ap = bass.AP(
                pads,
                i * ROWI + b * XB * D,
                [[PCOL, 112], [D, XB], [D, KW], [1, D]],
            )
            if eng is nc.scalar:
                return eng.copy(out=out_ap, in_=in_ap)
            return eng.tensor_copy(out=out_ap, in_=in_ap)

        engines = {0: nc.vector, 3: nc.vector, 6: nc.vector,
                   1: nc.scalar, 4: nc.scalar,
                   2: nc.gpsimd, 5: nc.gpsimd}

        n_st = 0
        for b in range(NB):
            h = b % 2
            for eng in (nc.vector, nc.scalar, nc.gpsimd):
                if b == 0:
                    eng.wait_ge(ld, n_ld)
                if b >= 2:
                    eng.wait_ge((st0, st1)[b % 2], 32 * (b // 2))
            for i in range(KH):
                eng = engines[i]
                sem = {id(nc.vector): cpv, id(nc.scalar): cps,
                       id(nc.gpsimd): cpg}[id(eng)]
                _copy(eng, b, h, i).then_inc(sem, 1)
            nc.sync.wait_ge(cpv, 3 * (b + 1))
            nc.sync.wait_ge(cps, 2 * (b + 1))
            nc.sync.wait_ge(cpg, 2 * (b + 1))
            for im in range(IMGS_PER_CORE):
                nc.sync.dma_start(
                    out=bass.AP(
                        out,
                        im * IMG_OUT + b * XB * K * D,
                        [[W * K * D, H], [1, OBH]],
                    ),
                    in_=bass.AP(
                        ob,
                        BASES[im] * OBROW + h * OBH,
                        [[OBROW, H], [1, OBH]],
                    ),
                ).then_inc((st0, st1)[h], 16)
                n_st += 16
        nc.sync.wait_ge(st0, 32 * (NB // 2))
        nc.sync.wait_ge(st1, 32 * (NB // 2))
    return nc


def kernel(x, height=48, width=48):
    from concourse.bass_utils import run_bass_kernel_spmd

    x = np.asarray(x)
    b, nh = x.shape[0], x.shape[1]
    xi = np.ascontiguousarray(x.reshape(b * nh, N, D))
    in_maps = [
        {"x": np.ascontiguousarray(xi[IMGS_PER_CORE * c: IMGS_PER_CORE * (c + 1)])}
        for c in range(N_CORES)
    ]
    if "nc" not in _CACHE:
        _CACHE["nc"] = _build_nc()
    res = run_bass_kernel_spmd(_CACHE["nc"], in_maps, core_ids=list(range(N_CORES)))
    y = np.stack([res.results[c]["out"] for c in range(N_CORES)])
    return y.reshape(b, nh, N, K, D).astype(np.float32, copy=False)


# revision 7
# speedup vs baseline: 1.0159x; 1.0159x over previous
"""LocalExpansion (7x7 unfold) Trainium2 Bass kernel — v2.

Full input x: [2, 8, 2304, 64] f32; output [2, 8, 2304, 49, 64] f32:
out[b,h,y*48+x,i*7+j,:] = img[b,h,y+i-3,x+j-3,:], zero outside.

16 images, 2 per core. v1 (pure DMA from padded rows) hit 345us: the
output's contiguous runs per (y,x,i) are only 1792 B, and ~32K small
descriptors at ~128 ns each cap the 16 SDMA engines at ~14 GB/s.

v2 buys big descriptors with a staged pipeline:
1. Load the image 7x from HBM (once per filter row i), each copy
   placed so padded row y+i sits on partition y (compute engines
   cannot read partition-shifted, so the shift is baked into the
   load). Extra HBM reads: 8.3 MB vs 57.8 MB of writes.
2. Compute engines (vector+scalar+gpsimd) build output-layout tiles:
   ob[y][x'*3136 + i*448 + j*64 + d] <- pads_i[y][(x+j)*64+d] — a
   free-dim overlapping-window copy, both images at once on 112
   partitions (imgA rows on 0-47, imgB on 64-111).
3. Out-DMA per x-block: one descriptor per partition = 3*49*64 f32
   = 37.6 KB contiguous both sides -> near line rate, HBM-bound.
Double-buffered x-blocks (XB=3, 16 blocks) overlap 2 and 3.
"""

import numpy as np

KH, KW = 7, 7
H, W, D = 48, 48, 64
N = H * W                      # 2304
K = KH * KW                    # 49
IMG_OUT = N * K * D            # 7,225,344 floats per image output
ROWI = (W + 6) * D             # 3456 floats per padded row
PCOL = KH * ROWI               # 24192 floats/partition in "pads"
XB = 3                         # x positions per output block
NB = W // XB                   # 16 blocks
OBH = XB * K * D               # 9408 floats: one buffer half per partition
OBROW = 2 * OBH                # 18816 floats/partition in "ob"
IMGS_PER_CORE = 2
N_CORES = 8
BASES = (0, 64)                # partition bases per image

_CACHE = {}


def _build_nc():
    import concourse.bass as bass
    import concourse.mybir as mybir

    nc = bass.Bass(trn_type="TRN2")
    x = nc.dram_tensor("x", [IMGS_PER_CORE, N, D], mybir.dt.float32,
                       kind="ExternalInput")
    out = nc.dram_tensor("out", [IMGS_PER_CORE, N, K, D], mybir.dt.float32,
                         kind="ExternalOutput")

    with (
        nc.sbuf_tensor("pads", [128, PCOL], mybir.dt.float32) as pads,
        nc.sbuf_tensor("ob", [128, OBROW], mybir.dt.float32) as ob,
        nc.semaphore("ms") as ms,
        nc.semaphore("ld") as ld,
        nc.semaphore("cpv") as cpv,
        nc.semaphore("cps") as cps,
        nc.semaphore("cpg") as cpg,
        nc.semaphore("st0") as st0,
        nc.semaphore("st1") as st1,
    ):
        # Zero "pads" (pad strips + junk partitions 48-63 read as 0).
        # Split across the three copy engines.
        third = PCOL // 3
        nc.vector.memset(bass.AP(pads, 0, [[PCOL, 128], [1, third]]), 0.0
                         ).then_inc(ms, 1)
        nc.gpsimd.memset(bass.AP(pads, third, [[PCOL, 128], [1, third]]), 0.0
                         ).then_inc(ms, 1)
        nc.vector.memset(bass.AP(pads, 2 * third, [[PCOL, 128], [1, third]]),
                         0.0).then_inc(ms, 1)

        # 14 shifted loads: pads[:, i-block] holds padded rows y+i on
        # partition y. Image rows g land on partition g+3-i.
        nc.sync.wait_ge(ms, 3)
        for im in range(IMGS_PER_CORE):
            bp = BASES[im]
            for i in range(KH):
                g_lo = max(0, i - 3)
                g_hi = min(H, i + 45)
                n_g = g_hi - g_lo
                p0 = g_lo + 3 - i
                nc.sync.dma_start(
                    out=bass.AP(
                        pads,
                        (bp + p0) * PCOL + i * ROWI + 3 * D,
                        [[PCOL, n_g], [1, W * D]],
                    ),
                    in_=bass.AP(
                        x,
                        im * N * D + g_lo * W * D,
                        [[W * D, n_g], [1, W * D]],
                    ),
                ).then_inc(ld, 16)
        n_ld = IMGS_PER_CORE * KH * 16

        # copy i -> engine: vector {0,3,6}, scalar {1,4}, gpsimd {2,5}
        def _copy(eng, b, h, i):
            out_ap = bass.AP(
                ob,
                h * OBH + i * KW * D,
                [[OBROW, 112], [K * D, XB], [1, KW * D]],
            )
            in_ap = bass.AP(
                pads,
                i * ROWI + b * XB * D,
                [[PCOL, 112], [D, XB], [1, KW * D]],
            )
            if eng is nc.scalar:
                return eng.copy(out=out_ap, in_=in_ap)
            return eng.tensor_copy(out=out_ap, in_=in_ap)

        engines = {0: nc.vector, 3: nc.vector,
                   1: nc.scalar, 4: nc.scalar, 6: nc.scalar,
                   2: nc.gpsimd, 5: nc.gpsimd}

        n_st = 0
        for b in range(NB):
            h = b % 2
            for eng in (nc.vector, nc.scalar, nc.gpsimd):
                if b == 0:
                    eng.wait_ge(ld, n_ld)
                if b >= 2:
                    eng.wait_ge((st0, st1)[b % 2], 32 * (b // 2))
            for i in range(KH):
                eng = engines[i]
                sem = {id(nc.vector): cpv, id(nc.scalar): cps,
                       id(nc.gpsimd): cpg}[id(eng)]
                _copy(eng, b, h, i).then_inc(sem, 1)
            nc.sync.wait_ge(cpv, 2 * (b + 1))
            nc.sync.wait_ge(cps, 3 * (b + 1))
            nc.sync.wait_ge(cpg, 2 * (b + 1))
            for im in range(IMGS_PER_CORE):
                nc.sync.dma_start(
                    out=bass.AP(
                        out,
                        im * IMG_OUT + b * XB * K * D,
                        [[W * K * D, H], [1, OBH]],
                    ),
                    in_=bass.AP(
                        ob,
                        BASES[im] * OBROW + h * OBH,
                        [[OBROW, H], [1, OBH]],
                    ),
                ).then_inc((st0, st1)[h], 16)
                n_st += 16
        nc.sync.wait_ge(st0, 32 * (NB // 2))
        nc.sync.wait_ge(st1, 32 * (NB // 2))
    return nc


def kernel(x, height=48, width=48):
    from concourse.bass_utils import run_bass_kernel_spmd

    x = np.asarray(x)
    b, nh = x.shape[0], x.shape[1]
    xi = np.ascontiguousarray(x.reshape(b * nh, N, D))
    in_maps = [
        {"x": np.ascontiguousarray(xi[IMGS_PER_CORE * c: IMGS_PER_CORE * (c + 1)])}
        for c in range(N_CORES)
    ]
    if "nc" not in _CACHE:
        _CACHE["nc"] = _build_nc()
    res = run_bass_kernel_spmd(_CACHE["nc"], in_maps, core_ids=list(range(N_CORES)))
    y = np.stack([res.results[c]["out"] for c in range(N_CORES)])
    return y.reshape(b, nh, N, K, D).astype(np.float32, copy=False)


# revision 9
# speedup vs baseline: 2.0129x; 1.9814x over previous
"""LocalExpansion (7x7 unfold) Trainium2 Bass kernel.

Full input x: [2, 8, 2304, 64] f32 (B=2, heads=8, N=48*48, D=64).
Full output:  [2, 8, 2304, 49, 64] f32 — out[b,h,y*W+x,i*7+j,:] =
x_img[b,h,y+i-3,x+j-3,:] with zero fill outside the 48x48 image.

Strategy (pure DMA, memory-regime):
- batch*heads = 16 images, 2 per core across 8 NeuronCores.
- Per core: zero-pad each 48x48x64 image into SBUF as 54 rows
  (one padded row per partition, 54*64 floats = 13824 B). Image 0 on
  partitions 0-53 (even-SDMA-engine half), image 1 on partitions
  64-117 (odd half) so concurrent DMAs load all 16 SDMA engines.
- For each filter row i (7 of them) one 3D DMA writes the whole
  [48 y, 48 x, 7*64 floats] slab: src is an overlapping sliding
  window (x stride 64 floats < 448-float element) read from SBUF,
  dst is strided DRAM with 1792 B contiguous chunks. Boundary zeros
  come for free from the padded SBUF image.
HBM traffic per core = 57.8 MB writes + 1.2 MB reads (~roofline).
"""

import numpy as np

KH, KW = 7, 7
H, W, D = 48, 48, 64
PH, PW = H + 6, W + 6          # 54x54 padded image
ROW = PW * D                   # floats per padded row (one SBUF partition)
N = H * W                      # 2304
K = KH * KW                    # 49
IMG_OUT = N * K * D            # floats per image output
IMGS_PER_CORE = 2
N_CORES = 8
BASES = (0, 64)                # SBUF base partitions per image

_CACHE = {}


def _build_nc():
    import concourse.bass as bass
    import concourse.mybir as mybir

    nc = bass.Bass(trn_type="TRN2")
    x = nc.dram_tensor("x", [IMGS_PER_CORE, N, D], mybir.dt.float32,
                       kind="ExternalInput")
    out = nc.dram_tensor("out", [IMGS_PER_CORE, N, K, D], mybir.dt.float32,
                         kind="ExternalOutput")

    with (
        nc.sbuf_tensor("pad", [128, ROW], mybir.dt.float32) as pad,
        nc.semaphore("ld") as ld,
        nc.semaphore("ms") as ms,
        nc.semaphore("st") as st,
    ):
        # Zero the whole padded buffer once (pad strips stay zero), then
        # load both images into the padded interiors.
        nc.vector.memset(
            bass.AP(pad, 0, [[ROW, 128], [1, ROW]]), 0.0
        ).then_inc(ms, 1)
        nc.sync.wait_ge(ms, 1)
        for im in range(IMGS_PER_CORE):
            bp = BASES[im]
            nc.sync.dma_start(
                out=bass.AP(pad, (bp + 3) * ROW + 3 * D, [[ROW, H], [1, W * D]]),
                in_=bass.AP(x, im * N * D, [[W * D, H], [1, W * D]]),
            ).then_inc(ld, 16)

        nc.sync.wait_ge(ld, IMGS_PER_CORE * 16)
        nc.scalar.wait_ge(ld, IMGS_PER_CORE * 16)
        nc.scalar.wait_ge(ms, 1)

        # 7 filter-row slabs per image; interleave images so both SDMA
        # engine halves (even: partitions 0-63, odd: 64-127) stay busy,
        # and alternate the two HWDGE rings (sync/scalar) per i.
        n_st = 0
        for i in range(KH):
            ring = nc.sync if i % 2 == 0 else nc.scalar
            for im in range(IMGS_PER_CORE):
                bp = BASES[im]
                ring.dma_start(
                    out=bass.AP(
                        out,
                        im * IMG_OUT + i * KW * D,
                        [[W * K * D, H], [K * D, W], [1, KW * D]],
                    ),
                    in_=bass.AP(
                        pad,
                        (bp + i) * ROW,
                        [[ROW, H], [D, W], [1, KW * D]],
                    ),
                ).then_inc(st, 16)
                n_st += 16
        nc.sync.wait_ge(st, n_st)
        nc.scalar.wait_ge(st, n_st)
    return nc


def kernel(x, height=48, width=48):
    from concourse.bass_utils import run_bass_kernel_spmd

    x = np.asarray(x)
    b, nh = x.shape[0], x.shape[1]
    xi = np.ascontiguousarray(x.reshape(b * nh, N, D))
    in_maps = [
        {"x": np.ascontiguousarray(xi[IMGS_PER_CORE * c: IMGS_PER_CORE * (c + 1)])}
        for c in range(N_CORES)
    ]
    if "nc" not in _CACHE:
        _CACHE["nc"] = _build_nc()
    res = run_bass_kernel_spmd(_CACHE["nc"], in_maps, core_ids=list(range(N_CORES)))
    y = np.stack([res.results[c]["out"] for c in range(N_CORES)])
    return y.reshape(b, nh, N, K, D).astype(np.float32, copy=False)
